# revision 39
# baseline (speedup 1.0000x reference)
"""GatedLinearRecurrence Trainium2 kernel (8-core SPMD, Bass/Tile).

Sharding: (batch=2) x (4 sequence chunks of 1024 tokens) across 8 cores.
Each core processes 1152 tokens: a 128-token warm-up window (re-computed
redundantly; worst-case recurrence carry decay over 128 tokens is ~1e-18,
so carry-in truncation is negligible) followed by its 1024 "main" tokens.
No collectives needed.

Precision plan (tolerance 2e-2; measured ~0.0178):
  - in_proj x+z halves and gate matmul: fp8e4m3 DoubleRow (two 128-row
    k-tiles per matmul -> ~1.9x bf16 throughput).  Weights scaled x16/x32
    on host; descale rides the PSUM-evacuation activation scale.
  - out_proj: bf16 (fp8 pushes rel-err past the 2e-2 gate).
  - elementwise chain in bf16; the recurrence scan keeps fp32 state.

Chunk-major software pipeline (the big structural idea): the 1152 tokens
split into 3 chunks of 384 processed as pipeline stages.  Stage c runs
LN/in_proj/conv/gate/scan for chunk c while the PREVIOUS chunk's
out_proj blocks interleave into the PE stream as hole-fillers.  This
mixes the ACT/DVE-heavy S1-S3 work with the PE-heavy gate/out_proj work
so no engine sits behind a phase wall (the phase-major layout left the
PE ~40% idle during LN/in_proj/conv and ACT/DVE ~70% idle during
out_proj).

Engine budget per stage (DVE is the pacer):
  DVE: LN stats + conv stt chains (1x-only op) + scans (~2.4 cyc/elem HW
       rate) + yg muls  ~= 61us total
  ACT: PSUM evacuations (x/z/gate/out), silus, sigmoids, fp8 casts
  Pool: DMA descriptor issue + bt muls + residual adds (NB: Pool shares
       its SBUF port with DVE, so only light duty is profitable; it
       cannot touch PSUM at all)
  PE:  DR matmuls + transposes + out_proj, kept warm (p-state!) by the
       interleaved out blocks.

Scheduling notes (learned from perfetto traces):
  - engines execute their instruction streams IN ORDER: emission order is
    the schedule.  DMA issues cost ~640ns of the issuing engine's time,
    so loads are merged (4-et w1 tiles, 2-et gate tiles, one consts
    pack) and ride the sync/gpsimd queues, never behind ACT compute.
  - conv is w0-normalized: taps hold r_k = w_k/w0 so the first stt uses
    xin itself as the unscaled in1 operand (3 stt instead of ts + 3 stt);
    w0 multiplies back via the silu's per-partition scale.
  - scan chunks chain across stages via the previous chunk's last ys
    column (fp32 state inside the scan instruction itself).
"""
import sys

for p in ("/opt/trn_rl_repo", "/root/.axon_site/_ro/trn_rl_repo"):
    if p not in sys.path:
        sys.path.insert(0, p)

import numpy as np
import ml_dtypes

import concourse.bass as bass
import concourse.bacc as bacc
import concourse.tile as tile
import concourse.mybir as mybir
from concourse.bass_utils import run_bass_kernel_spmd
from concourse.masks import make_identity

F32 = mybir.dt.float32
BF16 = mybir.dt.bfloat16
F8 = mybir.dt.float8e4
AF = mybir.ActivationFunctionType
OP = mybir.AluOpType
DR = mybir.MatmulPerfMode.DoubleRow

B, L, D = 2, 4096, 1024
DI = 2048            # d_inner
NT = 1152            # tokens per core (128 warm-up + 1024 main)
W = 128              # warm-up tokens
CHUNK = 1024
KD = D // 128        # 8 k-tiles over d_model
KC = DI // 128       # 16 k-tiles over d_inner
TC = 384             # chunk length (3 per core)
NTC = NT // TC
EPS = 1e-5
SG = 32.0            # fp8 gate weight scale

# consts pack column layout
C_CONVW, C_CONVSC, C_CONVB = 0, 48, 64
C_GATEB, C_GATEBN, C_INBX, C_INBZ = 80, 96, 112, 128
C_TOT = 144

_cache = {}


def _build():
    nc = bacc.Bacc(None, target_bir_lowering=False)

    x_h = nc.dram_tensor("x", [NT, D], F32, kind="ExternalInput")
    xbf_h = nc.dram_tensor("xbf", [NT, D], BF16, kind="ExternalInput")
    w1x_h = nc.dram_tensor("w1x", [KC, 128, KD * 128], F8, kind="ExternalInput")
    w1z_h = nc.dram_tensor("w1z", [KC, 128, KD * 128], F8, kind="ExternalInput")
    gw_h = nc.dram_tensor("gw", [KC, 128, KC * 128], F8, kind="ExternalInput")
    op_h = nc.dram_tensor("opw", [DI, D], BF16, kind="ExternalInput")
    cpk_h = nc.dram_tensor("cpk", [128, C_TOT], F32, kind="ExternalInput")
    mask_h = nc.dram_tensor("mask", [1, NT], BF16, kind="ExternalInput")
    out_h = nc.dram_tensor("out", [CHUNK, D], F32, kind="ExternalOutput")

    def merged_w1_ap(h, j):
        # 4 consecutive ets -> dest [128, 4, KD*128]
        X = KD * 128
        return bass.AP(tensor=h, offset=j * 4 * 128 * X,
                       ap=[[X, 128], [128 * X, 4], [1, X]])

    def merged_gw_ap(j):
        X = KC * 128
        return bass.AP(tensor=gw_h, offset=j * 2 * 128 * X,
                       ap=[[X, 128], [128 * X, 2], [1, X]])

    def merged_op_ap(j):
        return bass.AP(tensor=op_h, offset=j * 2 * 128 * D,
                       ap=[[D, 128], [128 * D, 2], [1, D]])

    with tile.TileContext(nc) as tc:
      with tc.tile_pool(name="consts", bufs=1) as consts:
        ident = consts.tile([128, 128], BF16, name="ident")
        make_identity(nc, ident)
        cpk = consts.tile([128, C_TOT], F32, name="cpk")
        nc.gpsimd.dma_start(out=cpk, in_=cpk_h.ap())
        mask_sb = consts.tile([128, W], BF16, name="mask_sb")
        nc.gpsimd.dma_start(
            out=mask_sb,
            in_=bass.AP(tensor=mask_h, offset=0, ap=[[0, 128], [1, W]]))
        eps_t = consts.tile([128, 1], F32, name="eps_t")
        nc.vector.memset(eps_t, EPS)

        convw = cpk[:, C_CONVW:C_CONVW + 48]
        convsc = cpk[:, C_CONVSC:C_CONVSC + 16]
        convb = cpk[:, C_CONVB:C_CONVB + 16]
        gateb = cpk[:, C_GATEB:C_GATEB + 16]
        gatebn = cpk[:, C_GATEBN:C_GATEBN + 16]
        inbx = cpk[:, C_INBX:C_INBX + 16]
        inbz = cpk[:, C_INBZ:C_INBZ + 16]

        with tc.tile_pool(name="xinp", bufs=1) as xinp, \
             tc.tile_pool(name="opwp", bufs=1) as opwp, \
             tc.tile_pool(name="lnr", bufs=2) as lnr, \
             tc.tile_pool(name="stat", bufs=4) as stp, \
             tc.tile_pool(name="xT8p", bufs=2) as xT8p, \
             tc.tile_pool(name="wr", bufs=2) as wr, \
             tc.tile_pool(name="gwr", bufs=2) as gwr, \
             tc.tile_pool(name="act1", bufs=1) as act1, \
             tc.tile_pool(name="roll", bufs=2) as roll, \
             tc.tile_pool(name="orow", bufs=2) as orow, \
             tc.tile_pool(name="pstr", bufs=2, space="PSUM") as pstr, \
             tc.tile_pool(name="psmm", bufs=2, space="PSUM") as psmm, \
             tc.tile_pool(name="psg", bufs=2, space="PSUM") as psg, \
             tc.tile_pool(name="pso", bufs=2, space="PSUM") as pso:

            # persistent per-et conv inputs (3 zero pad + NT columns)
            xin = [xinp.tile([128, NT + 3], BF16, name=f"xin{e}")
                   for e in range(KC)]
            for e in range(KC):
                nc.vector.memset(xin[e][:, 0:3], 0.0)
            # out_proj weights resident (bf16, 2-kt merged loads)
            opw = []
            for j in range(KC // 2):
                o = opwp.tile([128, 2, D], BF16, name=f"opw{j}")
                nc.gpsimd.dma_start(out=o, in_=merged_op_ap(j))
                opw.append(o)

            xT8 = {}          # chunk -> [128, KD, TC] fp8
            xc8 = {}          # chunk -> [128, KC, TC] fp8
            xcc = {}          # (et, c) -> [128, TC] bf16 silu(conv)
            szc = {}          # (et, c) -> [128, TC] bf16 silu(z)
            ysc = {}          # (et, c) -> [128, TC] bf16 scan out
            ygt = {}          # (et, c) -> [128, TC] bf16 y*silu(z)

            def ln_tile(it):
                c = it // 3
                col = (it % 3) * 128
                xt = lnr.tile([128, D], BF16, tag="xt", bufs=3, name="xt")
                nc.sync.dma_start(
                    out=xt, in_=xbf_h.ap()[it * 128:(it + 1) * 128, :])
                stats = stp.tile([128, 2, 6], F32, tag="stats", name="stats")
                for qi in range(2):
                    nc.vector.bn_stats(out=stats[:, qi, :],
                                       in_=xt[:, qi * 512:(qi + 1) * 512])
                mv = stp.tile([128, 2], F32, tag="mv", name="mv")
                nc.vector.bn_aggr(out=mv, in_=stats)
                rstd = stp.tile([128, 1], F32, tag="rstd", name="rstd")
                nc.scalar.activation(out=rstd, in_=mv[:, 1:2], func=AF.Sqrt,
                                     bias=eps_t, scale=1.0)
                nc.vector.reciprocal(out=rstd, in_=rstd)
                xhat = lnr.tile([128, D], BF16, tag="xhat", bufs=3, name="xhat")
                nc.vector.tensor_scalar(out=xhat, in0=xt, scalar1=mv[:, 0:1],
                                        scalar2=rstd, op0=OP.subtract,
                                        op1=OP.mult)
                for dp in range(KD // 2):
                    pst = pstr.tile([128, 2, 128], BF16, tag="tr", name="pst")
                    nc.tensor.transpose(
                        pst[:, 0, :], xhat[:, dp * 256:dp * 256 + 128], ident)
                    nc.tensor.transpose(
                        pst[:, 1, :], xhat[:, dp * 256 + 128:dp * 256 + 256],
                        ident)
                    ev = nc.vector.tensor_copy if dp < 1 else (
                        lambda out, in_: nc.scalar.copy(out=out, in_=in_))
                    ev(out=xT8[c][:, dp * 2:dp * 2 + 2, col:col + 128],
                       in_=pst)

            def in_mm(et, c, w4, bias_cols, is_z):
                ps = psmm.tile([128, TC], F32, tag="mm", name="ps")
                for kp in range(KD // 2):
                    nc.tensor.matmul(
                        ps, w4[et // 4][:, et % 4, 2 * kp:2 * kp + 2, :],
                        xT8[c][:, 2 * kp:2 * kp + 2, :],
                        start=(kp == 0), stop=(kp == KD // 2 - 1),
                        perf_mode=DR)
                if not is_z:
                    nc.scalar.activation(
                        out=xin[et][:, 3 + c * TC: 3 + (c + 1) * TC],
                        in_=ps, func=AF.Identity,
                        bias=bias_cols[:, et:et + 1], scale=1.0 / 16.0)
                else:
                    sz = roll.tile([128, TC], BF16, tag="sz", bufs=16,
                                   name="sz")
                    nc.scalar.activation(
                        out=sz, in_=ps, func=AF.Silu,
                        bias=bias_cols[:, et:et + 1], scale=1.0 / 16.0)
                    szc[et, c] = sz

            def conv(et, c):
                base = c * TC
                tmp = roll.tile([128, TC], BF16, tag="ctmp", bufs=2,
                                name="ctmp")
                nc.vector.scalar_tensor_tensor(
                    out=tmp, in0=xin[et][:, base + 1:base + 1 + TC],
                    scalar=convw[:, et * 3:et * 3 + 1],
                    in1=xin[et][:, base:base + TC], op0=OP.mult, op1=OP.add)
                for k in range(2, 4):
                    nc.vector.scalar_tensor_tensor(
                        out=tmp, in0=xin[et][:, base + k:base + k + TC],
                        scalar=convw[:, et * 3 + k - 1:et * 3 + k],
                        in1=tmp, op0=OP.mult, op1=OP.add)
                xc = roll.tile([128, TC], BF16, tag="xcc", bufs=17, name="xcc")
                if c == 0:
                    nc.scalar.activation(
                        out=xc[:, W:], in_=tmp[:, W:], func=AF.Silu,
                        bias=convb[:, et:et + 1], scale=convsc[:, et:et + 1])
                    tsw = stp.tile([128, W], BF16, tag="tsw", name="tsw")
                    nc.scalar.activation(
                        out=tsw, in_=tmp[:, 0:W], func=AF.Silu,
                        bias=convb[:, et:et + 1], scale=convsc[:, et:et + 1])
                    nc.vector.tensor_mul(xc[:, 0:W], tsw, mask_sb)
                else:
                    nc.scalar.activation(
                        out=xc, in_=tmp, func=AF.Silu,
                        bias=convb[:, et:et + 1], scale=convsc[:, et:et + 1])
                xcc[et, c] = xc
                nc.scalar.copy(out=xc8[c][:, et, :], in_=xc)

            def gate(et, c, gw2):
                ps = psg.tile([128, TC], F32, tag="g", name="psg")
                for kp in range(KC // 2):
                    nc.tensor.matmul(
                        ps, gw2[et // 2][:, et % 2, 2 * kp:2 * kp + 2, :],
                        xc8[c][:, 2 * kp:2 * kp + 2, :],
                        start=(kp == 0), stop=(kp == KC // 2 - 1),
                        perf_mode=DR)
                a_t = roll.tile([128, TC], BF16, tag="a", bufs=3, name="a")
                nc.scalar.activation(out=a_t, in_=ps, func=AF.Sigmoid,
                                     bias=gateb[:, et:et + 1], scale=1.0 / SG)
                am1 = roll.tile([128, TC], BF16, tag="am1", bufs=3, name="am1")
                nc.scalar.activation(out=am1, in_=ps, func=AF.Sigmoid,
                                     bias=gatebn[:, et:et + 1],
                                     scale=-1.0 / SG)
                bt = roll.tile([128, TC], BF16, tag="bt", bufs=3, name="bt")
                # bt on Pool: DVE is the global pacer (light Pool duty only
                # - its SBUF port is shared with DVE)
                nc.gpsimd.tensor_mul(bt, am1, xcc[et, c])
                ys = roll.tile([128, TC], BF16, tag="ys", bufs=17, name="ys")
                init = 0.0 if c == 0 else ysc[et, c - 1][:, TC - 1:TC]
                nc.vector.tensor_tensor_scan(
                    out=ys, data0=a_t, data1=bt, initial=init,
                    op0=OP.mult, op1=OP.add)
                ysc[et, c] = ys
                yg = roll.tile([128, TC], BF16, tag="yg", bufs=17, name="yg")
                lo = W if c == 0 else 0
                nc.vector.tensor_mul(yg[:, lo:], ys[:, lo:],
                                     szc[et, c][:, lo:])
                ygt[et, c] = yg

            def out_block(c, blk):
                # token block blk of chunk c (within-chunk col, 128 wide)
                lo = W if c == 0 else 0
                col = lo + blk * 128
                row = c * TC + col - W          # main-token row in out
                xres = orow.tile([128, D], F32, tag="xres", name="xres")
                nc.gpsimd.dma_start(
                    out=xres, in_=x_h.ap()[W + row:W + row + 128, :])
                for nb in range(2):
                    ps = pso.tile([128, 512], F32, tag="o", name="pso")
                    for kt in range(KC):
                        nc.tensor.matmul(
                            ps, ygt[kt, c][:, col:col + 128],
                            opw[kt // 2][:, kt % 2, nb * 512:(nb + 1) * 512],
                            start=(kt == 0), stop=(kt == KC - 1))
                    ev = roll.tile([128, 512], F32, tag="ev", bufs=2,
                                   name="ev")
                    nc.scalar.activation(out=ev, in_=ps, func=AF.Identity,
                                         bias=0.0, scale=1.0)
                    oh = roll.tile([128, 512], F32, tag="oh", bufs=2,
                                   name="oh")
                    nc.gpsimd.tensor_add(
                        oh, ev, xres[:, nb * 512:(nb + 1) * 512])
                    (nc.sync if (blk + nb) % 2 == 0 else nc.gpsimd).dma_start(
                        out=out_h.ap()[row:row + 128,
                                       nb * 512:(nb + 1) * 512],
                        in_=oh)

            def nblocks(c):
                return 2 if c == 0 else 3

            # ---------------- the 3-stage pipeline ----------------
            for c in range(NTC):
                xT8[c] = xT8p.tile([128, KD, TC], F8, tag="xT8",
                                   name=f"xT8_{c}")
                xc8[c] = xT8p.tile([128, KC, TC], F8, tag="xc8",
                                   name=f"xc8_{c}")
                for it in range(3 * c, 3 * c + 3):
                    ln_tile(it)
                # merged weight loads for this stage (pure-DMA queues)
                w1x4, w1z4 = [], []
                for j in range(KC // 4):
                    wt = wr.tile([128, 4, KD, 128], F8, tag="w1x", bufs=3,
                                 name=f"w1x{j}")
                    nc.gpsimd.dma_start(out=wt, in_=merged_w1_ap(w1x_h, j))
                    w1x4.append(wt)
                for et in range(KC):
                    in_mm(et, c, w1x4, inbx, False)
                if c > 0:
                    out_block(c - 1, 0)
                for j in range(KC // 4):
                    wt = wr.tile([128, 4, KD, 128], F8, tag="w1z", bufs=2,
                                 name=f"w1z{j}")
                    nc.sync.dma_start(out=wt, in_=merged_w1_ap(w1z_h, j))
                    w1z4.append(wt)
                for et in range(KC):
                    in_mm(et, c, w1z4, inbz, True)
                if c > 0:
                    out_block(c - 1, 1)
                gw2 = []
                for j in range(KC // 2):
                    gt = gwr.tile([128, 2, KC, 128], F8, tag="gw", bufs=3,
                                  name=f"gw{j}")
                    nc.sync.dma_start(out=gt, in_=merged_gw_ap(j))
                    gw2.append(gt)
                for et in range(KC):
                    conv(et, c)
                if c > 0 and nblocks(c - 1) > 2:
                    out_block(c - 1, 2)
                for et in range(KC):
                    gate(et, c, gw2)
            for blk in range(nblocks(NTC - 1)):
                out_block(NTC - 1, blk)

    nc.compile()
    return nc


def _prep_host(x, norm_w, norm_b, in_proj_w, conv_w, conv_b, gate_w, gate_b,
               out_proj_w):
    w1 = (in_proj_w * norm_w[None, :]).astype(np.float32)
    inb = (w1 @ norm_b.astype(np.float32)).astype(np.float32)   # [2*DI]

    def rearr(wT, dt, scale=1.0):
        # wT: [K, DI] -> per et slice [K, 128] -> [128, K//128, 128]
        k = wT.shape[0]
        out = np.empty((KC, 128, (k // 128) * 128), dt)
        for et in range(KC):
            s = (wT[:, et * 128:(et + 1) * 128] * scale).astype(dt)
            out[et] = s.reshape(k // 128, 128, 128).transpose(1, 0, 2).reshape(128, -1)
        return np.ascontiguousarray(out)

    w1xT = np.ascontiguousarray(w1[:DI].T)           # [D, DI]
    w1zT = np.ascontiguousarray(w1[DI:].T)           # [D, DI]
    w1x_r = rearr(w1xT, ml_dtypes.float8_e4m3, 16.0)
    w1z_r = rearr(w1zT, ml_dtypes.float8_e4m3, 16.0)
    gw_r = rearr(np.ascontiguousarray(gate_w.T), ml_dtypes.float8_e4m3, SG)
    op_r = np.ascontiguousarray(out_proj_w.T.astype(ml_dtypes.bfloat16))  # [DI, D]

    # w0-normalized conv taps: ratios r_k = w_k/w0 ride the stt chain, w0
    # multiplies back as the silu's per-partition scale
    cw = conv_w.reshape(DI, 4)
    w0 = cw[:, 0].copy()
    w0 = np.where(np.abs(w0) < 1e-8, 1e-8, w0)
    ratios = cw[:, 1:4] / w0[:, None]                # [DI, 3]

    def colpack(v):
        return v.reshape(KC, 128).T                  # [128, KC]

    cpk = np.zeros((128, C_TOT), np.float32)
    cpk[:, C_CONVW:C_CONVW + 48] = (
        ratios.reshape(KC, 128, 3).transpose(1, 0, 2).reshape(128, KC * 3))
    cpk[:, C_CONVSC:C_CONVSC + 16] = colpack(w0)
    cpk[:, C_CONVB:C_CONVB + 16] = colpack(conv_b)
    cpk[:, C_GATEB:C_GATEB + 16] = colpack(gate_b)
    cpk[:, C_GATEBN:C_GATEBN + 16] = colpack(-gate_b)
    cpk[:, C_INBX:C_INBX + 16] = colpack(inb[:DI])
    cpk[:, C_INBZ:C_INBZ + 16] = colpack(inb[DI:])
    cpk = np.ascontiguousarray(cpk)

    in_maps = []
    for core in range(8):
        b, j = core // 4, core % 4
        xs = np.zeros((NT, D), np.float32)
        start = j * CHUNK - W
        mask = np.ones((1, NT), ml_dtypes.bfloat16)
        if j == 0:
            xs[W:] = x[b, 0:CHUNK]
            mask[0, :W] = 0.0
        else:
            xs[:] = x[b, start:start + NT]
        in_maps.append({
            "x": np.ascontiguousarray(xs),
            "xbf": np.ascontiguousarray(xs.astype(ml_dtypes.bfloat16)),
            "w1x": w1x_r, "w1z": w1z_r,
            "gw": gw_r, "opw": op_r, "cpk": cpk, "mask": mask,
        })
    return in_maps


def kernel(x, norm_w, norm_b, in_proj_w, conv_w, conv_b, gate_w, gate_b,
           out_proj_w, _trace=False, _collect=None):
    x = np.asarray(x, np.float32)
    if "nc" not in _cache:
        _cache["nc"] = _build()
    nc = _cache["nc"]
    in_maps = _prep_host(
        x, np.asarray(norm_w, np.float32), np.asarray(norm_b, np.float32),
        np.asarray(in_proj_w, np.float32), np.asarray(conv_w, np.float32),
        np.asarray(conv_b, np.float32), np.asarray(gate_w, np.float32),
        np.asarray(gate_b, np.float32), np.asarray(out_proj_w, np.float32))
    res = run_bass_kernel_spmd(nc, in_maps, core_ids=list(range(8)), trace=_trace)
    if _collect is not None:
        _collect.append(res)
    out = np.empty((B, L, D), np.float32)
    for core in range(8):
        b, j = core // 4, core % 4
        out[b, j * CHUNK:(j + 1) * CHUNK] = res.results[core]["out"]
    return out
